# revision 1
# baseline (speedup 1.0000x reference)
"""Trainium2 Bass kernel for the 4-layer dendritic-LIF SNN (v3).

Strategy: data-parallel over batch (128 -> 16 per core, 8 cores, no
collectives).  All layer matmuls are batched over the full (T=100) x (Bc=16)
column set; only the elementwise LIF recurrences are sequential in time.

v3 design (sim: v1 1.14 ms -> v2 0.82 ms -> v3 target ~0.55 ms):
 - (1-beta)*(1-alpha) row scales and biases folded into the weights on host;
   the bias rides a ones-row in the contraction (free for L1 via the input
   padding; one extra 128-row contraction block for L2/L3).
 - float32r matmuls everywhere with 400-wide moving dim (1 cycle/row).
 - One (128,400)-wide dendrite scan per PSUM tile, reading PSUM directly;
   the beta table is zeroed at each t=0 column so the scan carry resets at
   batch-sample boundaries (4 samples per 400-col tile).
 - Branch-sum over K=4 as running tensor_tensor adds on the otherwise-idle
   GpSimd(Pool) engine, final add writing straight into the t-major ds buf.
 - Membrane recurrence: 3 ops/step via scalar_tensor_tensor (spike is
   computed transiently from the raw membrane trace each step); raw membrane
   values land in the spike tiles and one bulk in-place is_gt per engine
   side converts them to spikes.  DVE and GpSimd each own a contiguous range
   of o-blocks in SEPARATE tiles so the two serial chains run in parallel
   (shared tiles would false-serialize via tile-granular dep tracking).
 - Weights DMA'd one blocked transfer per 128-column output tile, x in 22
   per-k-tile DMAs: HWDGE issue cost ~650ns/DMA.

Toolchain workarounds (empirically validated in v1):
 - instructions may carry at most 1 sem-wait -> split extras onto NOPs
 - tensor_tensor_scan operands are kept full tiles; `initial` kept an AP
"""
import os
import sys
import time

import numpy as np

for _p in ("/root/.axon_site/_ro/trn_rl_repo", "/opt/trn_rl_repo"):
    if os.path.isdir(_p) and _p not in sys.path:
        sys.path.append(_p)

import concourse.bass as bass
import concourse.mybir as mybir
import concourse.tile as tile_mod
from concourse.tile import TileContext
from concourse.vector_clock import ScopedClock

f32 = mybir.dt.float32
f32r = mybir.dt.float32r
AL = mybir.AluOpType
AF = mybir.ActivationFunctionType

# ---------------------------------------------------------------- problem dims
B, T, IN, K = 128, 100, 2752, 4
INP = 2816              # IN padded to 22*128 (row 2752 = ones for bias fold)
H1, H2, H3, NCLS = 512, 512, 256, 100
NCORES = 8
BC = B // NCORES        # 16 samples per core
NCOL = BC * T           # 1600 matmul columns, col = b*T + t
QW = 400                # columns per PSUM tile = 4 samples x T

NOB = {1: 4, 2: 4, 3: 2}

# ------------------------------------------------------- tile workarounds
_MAX_WAITS = 1

_orig_lower = tile_mod.TileContext._lower_ordered_insts


def _split_waits_in_dict(nc, ordered):
    for bb_name, insts in ordered.items():
        new_list = []
        changed = False
        for inst in insts:
            si = inst.sync_info
            if si is not None and len(si.on_wait) > _MAX_WAITS:
                changed = True
                waits = list(si.on_wait)
                keep, extra = waits[:_MAX_WAITS], waits[_MAX_WAITS:]
                for w in extra:
                    nop = mybir.InstNoOp(
                        name=nc.get_next_instruction_name(), ins=[], outs=[]
                    )
                    nop.engine = inst.engine
                    nop.sync_info = mybir.SyncInfo(on_wait=[w], on_update=[])
                    nc.register_instruction(nop, overwrite=True)
                    new_list.append(nop)
                inst.sync_info = mybir.SyncInfo(
                    on_wait=keep, on_update=list(si.on_update)
                )
            new_list.append(inst)
        if changed:
            insts[:] = new_list


def _patched_lower(self, ordered):
    _split_waits_in_dict(self.nc, ordered)
    return _orig_lower(self, ordered)


def _patched_drain_and_barrier(self, tick_clock, wait_clock):
    drain_inst = self.nc.sync.drain()
    wait_clock.add_sem_waits(
        drain_inst.ins, ScopedClock({None: tick_clock.global_clock})
    )
    si = drain_inst.ins.sync_info
    if si is not None and len(si.on_wait) > 1:
        waits = list(si.on_wait)
        drain_inst.ins.sync_info = mybir.SyncInfo(
            on_wait=[waits[0]], on_update=list(si.on_update)
        )
        for w in waits[1:]:
            n2 = self.nc.sync.nop()
            n2.ins.sync_info = mybir.SyncInfo(on_wait=[w], on_update=[])
    self.nc.all_engine_barrier()
    popped = self.nc._tile_sem_poison_stack.pop()
    assert popped is self._sem_poison
    self.nc.clear_and_free_semaphores(list(self.sems.allocated().values()))
    self.nc.all_engine_barrier()


tile_mod.TileContext._lower_ordered_insts = _patched_lower
tile_mod.TileContext._drain_and_barrier = _patched_drain_and_barrier


# ---------------------------------------------------------------- the program
def _build_program():
    nc = bass.Bass()

    def din(name, shape):
        return nc.dram_tensor(name, shape, f32, kind="ExternalInput")

    xb = din("xb", [22, 128, NCOL])          # x blocked by contraction tile
    w1b = din("w1b", [16, INP, 128])         # blocked weights, col tile major
    w2b = din("w2b", [16, 640, 128])         # 640 = 4*128 spk rows + ones blk
    w3b = din("w3b", [8, 640, 128])
    bt1 = din("bt1", [16, 128, QW])          # scan beta tables (0 at t=0 col)
    bt2 = din("bt2", [16, 128, QW])
    bt3 = din("bt3", [8, 128, QW])
    altab1 = din("altab1", [128, 64])        # alpha bcast over (o_hi, b)
    altab2 = din("altab2", [128, 64])
    altab3 = din("altab3", [128, 32])
    onesd = din("onesd", [128, NCOL])        # ones block for L2/L3 bias row
    mem01 = din("mem01", [128, 64])
    mem02 = din("mem02", [128, 64])
    mem03 = din("mem03", [128, 32])
    w4T = din("w4T", [H3, NCLS])
    b4c = din("b4c", [NCLS, 1])
    out = nc.dram_tensor("out", [NCLS, BC], f32, kind="ExternalOutput")

    with TileContext(nc) as tc:
        with (
            tc.tile_pool(name="const", bufs=1) as cpool,
            tc.tile_pool(name="ds1p", bufs=1) as ds1p,
        ):
            zini = cpool.tile([128, 1], f32)
            nc.vector.memset(zini[:], 0.0)

            alt = {}
            for nm, dr, w in (
                ("altab1", altab1, 64), ("altab2", altab2, 64),
                ("altab3", altab3, 32),
            ):
                t_ = cpool.tile([128, w], f32, tag=nm)
                nc.sync.dma_start(out=t_[:], in_=dr[:])
                alt[nm] = t_

            # initial-membrane tiles, one per layer
            mem0t = {}
            for li, dr, w in ((1, mem01, 64), (2, mem02, 64), (3, mem03, 32)):
                t_ = cpool.tile([128, w], f32, tag=f"m{li}", name=f"m{li}")
                nc.sync.dma_start(out=t_[:], in_=dr[:])
                mem0t[li] = t_

            ds1 = ds1p.tile([128, T * 64], f32)

            # ---------------------------------------------------- layer pass
            def layer_pass(li, kt, n_ob, wdram, btdram, ds, rhs_of):
                """Matmul + dendrite scan + branch-sum for one layer.
                kt: contraction tiles, n_ob: H/128 output blocks,
                rhs_of(k, q) -> [128, QW] f32r AP."""
                o_hi_w = n_ob * 16
                with (
                    tc.tile_pool(name=f"w{li}", bufs=2) as wpool,
                    tc.tile_pool(name=f"bt{li}", bufs=2) as btpool,
                    tc.tile_pool(name=f"st{li}", bufs=1) as stg,
                    tc.tile_pool(name=f"ac{li}", bufs=1) as accp,
                    tc.tile_pool(name=f"mm{li}", bufs=2, space="PSUM") as mmps,
                ):
                    dsv = ds[:].rearrange("p (t c) -> p c t", c=o_hi_w)
                    for ob in range(n_ob):
                        accs = [None] * 4
                        for k4 in range(K):
                            m = k4 * n_ob + ob
                            w_ = wpool.tile([128, kt * 128], f32r, tag="w")
                            nc.sync.dma_start(
                                out=w_[:].rearrange(
                                    "p (k n) -> p k n", k=kt
                                ),
                                in_=wdram[m].rearrange(
                                    "(k p) n -> p k n", p=128
                                ).bitcast(f32r),
                            )
                            bt_ = btpool.tile([128, QW], f32, tag="bt")
                            nc.sync.dma_start(out=bt_[:], in_=btdram[m])
                            ps = [
                                mmps.tile(
                                    [128, QW], f32, tag=f"ps{q}",
                                    name=f"ps{q}",
                                )
                                for q in range(4)
                            ]
                            for k in range(kt):
                                lhs = w_[:, k * 128:(k + 1) * 128]
                                for q in range(4):
                                    nc.tensor.matmul(
                                        ps[q][:],
                                        lhs,
                                        rhs_of(k, q),
                                        start=(k == 0),
                                        stop=(k == kt - 1),
                                    )
                            for q in range(4):
                                if k4 == 0:
                                    accs[q] = accp.tile(
                                        [128, QW], f32, tag=f"a{q}",
                                        name=f"a{q}",
                                    )
                                    nc.vector.tensor_tensor_scan(
                                        out=accs[q][:],
                                        data0=bt_[:],
                                        data1=ps[q][:],
                                        initial=zini[:, 0:1],
                                        op0=AL.mult,
                                        op1=AL.add,
                                    )
                                else:
                                    st = stg.tile(
                                        [128, QW], f32, tag=f"s{q}",
                                        name=f"s{q}",
                                    )
                                    nc.vector.tensor_tensor_scan(
                                        out=st[:],
                                        data0=bt_[:],
                                        data1=ps[q][:],
                                        initial=zini[:, 0:1],
                                        op0=AL.mult,
                                        op1=AL.add,
                                    )
                                    if k4 < K - 1:
                                        nc.gpsimd.tensor_tensor(
                                            out=accs[q][:],
                                            in0=accs[q][:],
                                            in1=st[:],
                                            op=AL.add,
                                        )
                                    else:
                                        off = ob * 16 + q * 4
                                        nc.gpsimd.tensor_tensor(
                                            out=dsv[:, off:off + 4, :],
                                            in0=accs[q][:].rearrange(
                                                "p (b t) -> p b t", b=4
                                            ),
                                            in1=st[:].rearrange(
                                                "p (b t) -> p b t", b=4
                                            ),
                                            op=AL.add,
                                        )

            # ----------------------------------------------------- mem scan
            def mem_scan(li, ds, altab, trace, spk):
                """LIF membrane recurrence + spike threshold, on DVE.

                mem[t] = alpha*mem[t-1] + ds[t] - spk[t-1], spk = mem > 1.
                Raw membrane values are written into `trace` (layout (o b t),
                t contiguous); the spike enters the recurrence transiently
                via (mem>1) inside an STT each step (3 ops/step).  The bulk
                is_gt converting the trace to f32r spikes is split between
                DVE and GpSimd.
                """
                n_ob = NOB[li]
                o_hi_w = n_ob * 16
                tr4 = trace[:].rearrange(
                    "p (o b t) -> p o b t", o=n_ob, b=BC
                )
                m03 = mem0t[li][:].rearrange("p (o b) -> p o b", o=n_ob)
                alt3 = altab[:].rearrange("p (o b) -> p o b", o=n_ob)
                eng = nc.vector
                with tc.tile_pool(name=f"ms{li}", bufs=3) as msp:
                    for t in range(T):
                        ds3 = ds[:, t * o_hi_w:(t + 1) * o_hi_w].rearrange(
                            "p (o b) -> p o b", o=n_ob
                        )
                        cur = tr4[:, :, :, t]
                        if t == 0:
                            v = msp.tile([128, o_hi_w], f32, tag="v",
                                         name="v")
                            v3 = v[:].rearrange("p (o b) -> p o b", o=n_ob)
                            eng.tensor_tensor(
                                out=v3, in0=m03, in1=alt3, op=AL.mult
                            )
                            eng.tensor_tensor(
                                out=cur, in0=v3, in1=ds3, op=AL.add
                            )
                            continue
                        prev = tr4[:, :, :, t - 1]
                        w_ = msp.tile([128, o_hi_w], f32, tag="w", name="w")
                        w3 = w_[:].rearrange("p (o b) -> p o b", o=n_ob)
                        eng.scalar_tensor_tensor(
                            out=w3, in0=prev, scalar=1.0, in1=ds3,
                            op0=AL.is_gt, op1=AL.subtract,
                        )
                        v = msp.tile([128, o_hi_w], f32, tag="v", name="v")
                        v3 = v[:].rearrange("p (o b) -> p o b", o=n_ob)
                        eng.tensor_tensor(
                            out=v3, in0=prev, in1=alt3, op=AL.mult
                        )
                        eng.tensor_tensor(
                            out=cur, in0=v3, in1=w3, op=AL.subtract
                        )
                    # bulk spike conversion, split across DVE and GpSimd
                    half = (n_ob // 2) * NCOL
                    nc.vector.tensor_scalar(
                        out=spk[:, :half], in0=trace[:, :half],
                        scalar1=1.0, scalar2=None, op0=AL.is_gt,
                    )
                    nc.gpsimd.tensor_scalar(
                        out=spk[:, half:], in0=trace[:, half:],
                        scalar1=1.0, scalar2=None, op0=AL.is_gt,
                    )

            # -------------------------------------------------------- layer 1
            with tc.tile_pool(name="xp", bufs=1) as xpool:
                xts = []
                for k in range(22):
                    xt = xpool.tile([128, NCOL], f32r, tag=f"x{k}",
                                    name=f"x{k}")
                    nc.scalar.dma_start(out=xt[:], in_=xb[k].bitcast(f32r))
                    xts.append(xt)

                def l1_rhs(k, q):
                    return xts[k][:, q * QW:(q + 1) * QW]

                layer_pass(1, 22, 4, w1b, bt1, ds1, l1_rhs)

            with (
                tc.tile_pool(name="mid", bufs=1) as midp,
            ):
                spk1 = midp.tile([128, 4 * NCOL], f32r, tag="spk1")
                spk2 = midp.tile([128, 4 * NCOL], f32r, tag="spk2")
                spk3 = midp.tile([128, 2 * NCOL], f32r, tag="spk3")
                ds2 = midp.tile([128, T * 64], f32, tag="ds2")
                ds3t = midp.tile([128, T * 32], f32, tag="ds3t")
                ones = midp.tile([128, NCOL], f32r, tag="ones")
                nc.sync.dma_start(out=ones[:], in_=onesd[:].bitcast(f32r))

                def spk_rhs(spk, n_ob):
                    def f(k, q):
                        if k < n_ob:
                            return spk[:, k * NCOL + q * QW:
                                       k * NCOL + (q + 1) * QW]
                        return ones[:, q * QW:(q + 1) * QW]

                    return f

                def run_scan(li, ds_, spk):
                    n_ob = NOB[li]
                    with tc.tile_pool(name=f"tr{li}", bufs=1) as trp:
                        tra = trp.tile([128, n_ob * NCOL], f32,
                                       tag=f"tr{li}", name=f"tr{li}")
                        mem_scan(li, ds_, alt[f"altab{li}"], tra, spk)

                run_scan(1, ds1, spk1)

                # ---------------------------------------------------- layer 2
                layer_pass(2, 5, 4, w2b, bt2, ds2, spk_rhs(spk1, 4))
                run_scan(2, ds2, spk2)

                # ---------------------------------------------------- layer 3
                layer_pass(3, 5, 2, w3b, bt3, ds3t, spk_rhs(spk2, 4))
                run_scan(3, ds3t, spk3)

                # ---------------------------------------------------- layer 4
                with (
                    tc.tile_pool(name="l4", bufs=1) as l4p,
                    tc.tile_pool(name="l4ps", bufs=1, space="PSUM") as l4ps,
                ):
                    ps4 = l4ps.tile([NCLS, BC], f32)
                    for kk in range(H3 // 128):
                        red = l4p.tile([128, BC], f32, tag=f"red{kk}",
                                       name=f"red{kk}")
                        nc.vector.tensor_reduce(
                            out=red[:],
                            in_=spk3[
                                :, kk * NCOL:(kk + 1) * NCOL
                            ].bitcast(f32).rearrange(
                                "p (b t) -> p b t", b=BC
                            ),
                            axis=mybir.AxisListType.X,
                            op=AL.add,
                        )
                        w4_ = l4p.tile([128, NCLS], f32, tag=f"w4{kk}",
                                       name=f"w4{kk}")
                        nc.sync.dma_start(
                            out=w4_[:], in_=w4T[kk * 128:(kk + 1) * 128, :]
                        )
                        nc.tensor.matmul(
                            ps4[:], w4_[:], red[:],
                            start=(kk == 0), stop=(kk == H3 // 128 - 1),
                        )
                    b4sb = l4p.tile([NCLS, 1], f32)
                    nc.sync.dma_start(out=b4sb[:], in_=b4c[:])
                    osb = l4p.tile([NCLS, BC], f32)
                    nc.scalar.activation(
                        osb[:], ps4[:], AF.Identity,
                        bias=b4sb[:, 0:1], scale=1.0 / T,
                    )
                    nc.sync.dma_start(out=out[:], in_=osb[:])

    return nc


_NC_CACHE = None


def _get_program():
    global _NC_CACHE
    if _NC_CACHE is None:
        _NC_CACHE = _build_program()
    return _NC_CACHE


# ---------------------------------------------------------------- host prep
def _sigmoid(x):
    return 1.0 / (1.0 + np.exp(-np.asarray(x, np.float64)))


def _km(a, O):
    """(O*K,...) o-major rows -> k-major rows (K*O, ...)."""
    return a.reshape(O, K, *a.shape[1:]).transpose(
        1, 0, *range(2, a.ndim + 1)
    ).reshape(K * O, *a.shape[1:])


def _layer_tables(W, b, tau_m, tau_n, mask, O, in_dim, in_pad, n_m):
    """Blocked folded weights + scan beta tables + alpha table for one layer.

    Returns dict with:
      wb   (n_m, in_pad, 128)  folded weight blocks; row `in_dim` = bias row
      bt   (n_m, 128, QW)      scan multiplier, zeroed at each t=0 column
      altab (128, O//128*16)   alpha broadcast over (o_hi, b)
    """
    Wm = (W * mask).astype(np.float64)                 # (O*K, In), o-major
    Wkm = _km(Wm, O)                                   # k-major rows
    bkm = _km(np.asarray(b, np.float64), O)
    beta = _sigmoid(tau_n).reshape(O, K).T.reshape(-1)  # k-major (K*O,)
    alpha = _sigmoid(tau_m)                            # (O,)
    alpha_km = np.tile(alpha, K)                       # k-major row alpha
    s = (1.0 - beta) * (1.0 - alpha_km)                # row scale
    Wf = (s[:, None] * Wkm)                            # (K*O, In)
    bf = s * bkm                                       # (K*O,)

    wT = np.zeros((in_pad, K * O), np.float32)
    wT[:in_dim] = Wf.T.astype(np.float32)
    wT[in_dim] = bf.astype(np.float32)
    wb = np.stack(
        [wT[:, m * 128:(m + 1) * 128] for m in range(n_m)]
    )                                                  # (n_m, in_pad, 128)

    colmask = (np.arange(QW) % T != 0).astype(np.float32)  # 0 at t=0 cols
    btm = beta.astype(np.float32).reshape(n_m, 128)
    bt = btm[:, :, None] * colmask[None, None, :]      # (n_m, 128, QW)

    n_ob = O // 128
    altab = np.ascontiguousarray(
        np.repeat(
            alpha.astype(np.float32).reshape(n_ob, 128).T[:, :, None],
            BC, axis=2,
        ).reshape(128, n_ob * BC)
    )
    return dict(wb=np.ascontiguousarray(wb), bt=np.ascontiguousarray(bt),
                altab=altab)


def _mem0_rearrange(m0, O):
    # (BC, O) -> [128, n_ob*BC] with [p, o_hi*BC + b] = m0[b, o_hi*128+p]
    n_ob = O // 128
    return np.ascontiguousarray(
        m0.T.reshape(n_ob, 128, BC).transpose(1, 0, 2).reshape(128, n_ob * BC)
    ).astype(np.float32)


LAST_EXEC_NS = None

_EXEC_CACHE = None


def _get_exec():
    """Build (once) a cached jitted PJRT executable for the Bass program,
    mirroring concourse.bass2jax.run_bass_via_pjrt so repeat calls skip
    walrus compilation and can be timed."""
    global _EXEC_CACHE
    if _EXEC_CACHE is not None:
        return _EXEC_CACHE
    import jax
    from jax.sharding import Mesh, PartitionSpec
    from jax.experimental.shard_map import shard_map
    import concourse.mybir as _mb
    from concourse import bass2jax as b2j

    nc = _get_program()
    b2j.install_neuronx_cc_hook()
    partition_name = (
        nc.partition_id_tensor.name if nc.partition_id_tensor else None
    )
    in_names, out_names, out_avals, zero_outs = [], [], [], []
    for alloc in nc.m.functions[0].allocations:
        if not isinstance(alloc, _mb.MemoryLocationSet):
            continue
        name = alloc.memorylocations[0].name
        if alloc.kind == "ExternalInput":
            if name != partition_name:
                in_names.append(name)
        elif alloc.kind == "ExternalOutput":
            shape = tuple(alloc.tensor_shape)
            dtype = _mb.dt.np(alloc.dtype)
            out_names.append(name)
            out_avals.append(jax.core.ShapedArray(shape, dtype))
            zero_outs.append(np.zeros(shape, dtype))
    n_params = len(in_names)
    all_in_names = list(in_names) + list(out_names)
    if partition_name is not None:
        all_in_names.append(partition_name)
    donate = tuple(range(n_params, n_params + len(out_names)))

    def _body(*args):
        operands = list(args)
        if partition_name is not None:
            operands.append(b2j.partition_id_tensor())
        outs = b2j._bass_exec_p.bind(
            *operands,
            out_avals=tuple(out_avals),
            in_names=tuple(all_in_names),
            out_names=tuple(out_names),
            lowering_input_output_aliases=(),
            sim_require_finite=True,
            sim_require_nnan=True,
            nc=nc,
        )
        return tuple(outs)

    devices = jax.devices()[:NCORES]
    mesh = Mesh(np.asarray(devices), ("core",))
    in_specs = (PartitionSpec("core"),) * (n_params + len(out_names))
    out_specs = (PartitionSpec("core"),) * len(out_names)
    sharded = jax.jit(
        shard_map(
            _body, mesh=mesh, in_specs=in_specs, out_specs=out_specs,
            check_rep=False,
        ),
        donate_argnums=donate,
        keep_unused=True,
    )
    _EXEC_CACHE = (sharded, in_names, out_names, out_avals, zero_outs, mesh)
    return _EXEC_CACHE


def _run_on_device(in_maps, repeats=1):
    """Execute the cached program; returns (per-core outputs, best_exec_ns).
    Inputs are placed on-device once so repeat timings measure execution,
    not host->device transfer of the ~50MB/core weight set."""
    import jax
    from jax.sharding import NamedSharding, PartitionSpec

    sharded, in_names, out_names, out_avals, zero_outs, mesh = _get_exec()
    concat_in = [
        np.concatenate([in_maps[c][n] for c in range(NCORES)], axis=0)
        for n in in_names
    ]
    shd = NamedSharding(mesh, PartitionSpec("core"))
    dev_in = [jax.device_put(a, shd) for a in concat_in]
    for a in dev_in:
        a.block_until_ready()
    best = None
    out_arrs = None
    for _ in range(max(1, repeats)):
        concat_zeros = [
            jax.device_put(
                np.zeros((NCORES * z.shape[0], *z.shape[1:]), z.dtype), shd
            )
            for z in zero_outs
        ]
        for a in concat_zeros:
            a.block_until_ready()
        t0 = time.perf_counter()
        out_arrs = sharded(*dev_in, *concat_zeros)
        out_arrs = [np.asarray(a) for a in out_arrs]
        dt = time.perf_counter() - t0
        if best is None or dt < best:
            best = dt
    results = [
        {
            n: out_arrs[i].reshape(NCORES, *out_avals[i].shape)[c]
            for i, n in enumerate(out_names)
        }
        for c in range(NCORES)
    ]
    return results, int(best * 1e9)


def kernel(
    dvs_inp, W1, b1, tau_m1, tau_n1, mask1,
    W2, b2, tau_m2, tau_n2, mask2,
    W3, b3, tau_m3, tau_n3, mask3,
    W4, b4, mem1_0, mem2_0, mem3_0,
):
    global LAST_EXEC_NS
    nc = _get_program()

    t1 = _layer_tables(W1, b1, tau_m1, tau_n1, mask1, H1, IN, INP, 16)
    t2 = _layer_tables(W2, b2, tau_m2, tau_n2, mask2, H2, H1, 640, 16)
    t3 = _layer_tables(W3, b3, tau_m3, tau_n3, mask3, H3, H2, 640, 8)
    shared = {
        "w1b": t1["wb"], "w2b": t2["wb"], "w3b": t3["wb"],
        "bt1": t1["bt"], "bt2": t2["bt"], "bt3": t3["bt"],
        "altab1": t1["altab"], "altab2": t2["altab"], "altab3": t3["altab"],
        "onesd": np.ones((128, NCOL), np.float32),
        "w4T": np.ascontiguousarray(W4.T.astype(np.float32)),
        "b4c": np.ascontiguousarray(b4.astype(np.float32)[:, None]),
    }
    x_all = np.asarray(dvs_inp, np.float32).reshape(B, T, IN)
    in_maps = []
    for c in range(NCORES):
        b0 = c * BC
        xc = np.zeros((INP, NCOL), np.float32)
        xc[:IN] = x_all[b0:b0 + BC].transpose(2, 0, 1).reshape(IN, NCOL)
        xc[IN] = 1.0
        m = dict(shared)
        m["xb"] = np.ascontiguousarray(xc.reshape(22, 128, NCOL))
        m["mem01"] = _mem0_rearrange(np.asarray(mem1_0)[b0:b0 + BC], H1)
        m["mem02"] = _mem0_rearrange(np.asarray(mem2_0)[b0:b0 + BC], H2)
        m["mem03"] = _mem0_rearrange(np.asarray(mem3_0)[b0:b0 + BC], H3)
        in_maps.append(m)

    results, exec_ns = _run_on_device(
        in_maps, repeats=int(os.environ.get("KERNEL_REPEATS", "10"))
    )
    LAST_EXEC_NS = exec_ns

    out_full = np.empty((B, NCLS), np.float32)
    for c in range(NCORES):
        out_full[c * BC:(c + 1) * BC] = results[c]["out"].T
    return out_full

